# revision 1
# baseline (speedup 1.0000x reference)
"""4-bit column-block-quantized linear (ColBlockQuantizedLinear) on 8 TRN2 cores.

Math:  out[b,o] = scales[o] * (sum_i inp[b,i]*wq[o,i] - zeros[o]*rowsum[b])
where wq comes from packed bytes q[o,j] (j = i//2): even i -> low nibble,
odd i -> high nibble.

Device-side identity (all O(O*I) work stays on-device):
    sum_j l*a + sum_j h*b = sum_j q*a + sum_j h*(b-16a)
with q = 16h + l, a[j]=inp[:,2j], b[j]=inp[:,2j+1].

The h-stream never materializes h as an integer: a 4-instruction uint16
bit-trick on DVE writes the bf16 BIT PATTERN 0x4300|(h<<3) (= value 128+8h,
linear in h) at 4x DVE mode.  The matmul pairs it with c' = (b-16a)/8 and the
constant 128*sum(c') falls out as a rank-1 correction row.  The q-stream is a
plain u8->bf16 cast (exact, 0..255) split across ACT and GPSIMD.  Activations
are hi/lo bf16-split so the bf16 matmuls give ~fp32 accuracy; zeros*rowsum and
the 128-offset are a K=6 correction matmul with hi/lo-split factors.

Host byte layout: per core the packed bytes [2048, 1376] are column-paired as
(m, 688+m) into uint16 [2048, 688], so the bit-trick's two output streams land
contiguously in natural column order; the q-cast output is column-interleaved
and its matmuls read it through stride-2 APs.

Sharding: column-parallel over out_features (1376 rows/core), inputs
replicated; per-core output [16,1376] gathered on host.
"""

import numpy as np
import ml_dtypes

B = 16
I = 4096
O = 11008
NCORES = 8
OS = O // NCORES          # 1376 out-features per core
HOS = OS // 2             # 688, u16-packed column count
HALF = I // 2             # 2048 packed columns
KT = HALF // 128          # 16 contraction tiles
# psum-bank o-blocks, each a single arithmetic progression in the interleaved
# q-cast layout (no block crosses the 688-column half boundary)
BLKS = [(0, 512), (512, 176), (688, 512), (1200, 176)]
N_ACT_CAST = 10           # q-cast tiles on ACT; rest on GPSIMD

BF16 = ml_dtypes.bfloat16

_CACHE = {}


def _split_hi_lo(x64):
    """Split float64 array into (hi, lo) bf16 parts: hi+lo ~= x to ~2^-17."""
    hi = x64.astype(BF16)
    lo = (x64 - hi.astype(np.float64)).astype(BF16)
    return hi, lo


def _qcast_ap(qb, s, n):
    """Stride-2 AP over the interleaved q-cast tile covering natural columns
    [s, s+n) (s,n within one half)."""
    if s < HOS:
        return qb[:, 2 * s : 2 * (s + n) : 2]
    return qb[:, 2 * (s - HOS) + 1 : 2 * (s - HOS + n) : 2]


def _build_program():
    import concourse.bacc as bacc
    import concourse.mybir as mybir
    import concourse.tile as tile

    dt = mybir.dt
    op = mybir.AluOpType
    nc = bacc.Bacc("TRN2", target_bir_lowering=False)

    q = nc.dram_tensor("q", [HALF, HOS], dt.uint16, kind="ExternalInput")
    statA = nc.dram_tensor("statA", [128, KT * 64], dt.bfloat16, kind="ExternalInput")
    statC = nc.dram_tensor("statC", [128, KT * 64], dt.bfloat16, kind="ExternalInput")
    corrL = nc.dram_tensor("corrL", [6, 64], dt.bfloat16, kind="ExternalInput")
    corrR = nc.dram_tensor("corrR", [6, OS], dt.bfloat16, kind="ExternalInput")
    sc = nc.dram_tensor("sc", [B, OS], dt.float32, kind="ExternalInput")
    out = nc.dram_tensor("out", [B, OS], dt.float32, kind="ExternalOutput")

    with tile.TileContext(nc) as tc:
        with (
            tc.tile_pool(name="consts", bufs=1) as cpool,
            tc.tile_pool(name="qp", bufs=3) as qpool,
            tc.tile_pool(name="tp", bufs=2) as tpool,
            tc.tile_pool(name="wp", bufs=3) as wpool,
            tc.tile_pool(name="op", bufs=2) as opool,
            tc.tile_pool(name="ps", bufs=1, space="PSUM") as pspool,
        ):
            statA_sb = cpool.tile([128, KT * 64], dt.bfloat16, name="statA_sb")
            statC_sb = cpool.tile([128, KT * 64], dt.bfloat16, name="statC_sb")
            corrL_sb = cpool.tile([6, 64], dt.bfloat16, name="corrL_sb")
            corrR_sb = cpool.tile([6, OS], dt.bfloat16, name="corrR_sb")
            sc_sb = cpool.tile([B, OS], dt.float32, name="sc_sb")
            nc.sync.dma_start(statA_sb, statA[:, :])
            nc.sync.dma_start(statC_sb, statC[:, :])
            nc.sync.dma_start(corrL_sb, corrL[:, :])
            nc.sync.dma_start(corrR_sb, corrR[:, :])
            nc.sync.dma_start(sc_sb, sc[:, :])

            psums = [
                pspool.tile([64, n], dt.float32, name=f"ps{i}")
                for i, (s, n) in enumerate(BLKS)
            ]

            for kt in range(KT):
                qt = qpool.tile([128, HOS], dt.uint16, name="qt", tag="qt")
                nc.sync.dma_start(qt, q[kt * 128 : (kt + 1) * 128, :])
                qb = wpool.tile([128, OS], dt.bfloat16, name="qb", tag="qb")
                hb = wpool.tile([128, OS], dt.bfloat16, name="hb", tag="hb")
                hbu = hb.bitcast(dt.uint16)
                t1 = tpool.tile([128, HOS], dt.uint16, name="t1", tag="t1")
                t2 = tpool.tile([128, HOS], dt.uint16, name="t2", tag="t2")
                # q-cast (exact bf16 of 0..255); interleaved column order
                if kt < N_ACT_CAST:
                    nc.scalar.activation(
                        qb, qt.bitcast(dt.uint8), mybir.ActivationFunctionType.Copy
                    )
                else:
                    nc.gpsimd.tensor_copy(qb, qt.bitcast(dt.uint8))
                # h-stream bit trick: bf16 bits 0x4300|(h<<3) = 128+8h
                nc.vector.tensor_scalar(t1, qt, 1, None, op.logical_shift_right)
                nc.vector.tensor_scalar(
                    hbu[:, 0:HOS], t1, 0x78, 0x4300, op.bitwise_and, op.bitwise_or
                )
                nc.vector.tensor_scalar(
                    t2, t1, 8, 0x78, op.logical_shift_right, op.bitwise_and
                )
                nc.vector.tensor_scalar(
                    hbu[:, HOS:OS], t2, 0x4300, None, op.bitwise_or
                )
                for i, (s, n) in enumerate(BLKS):
                    nc.tensor.matmul(
                        psums[i],
                        statA_sb[:, kt * 64 : kt * 64 + 64],
                        _qcast_ap(qb, s, n),
                        start=(kt == 0),
                        stop=False,
                    )
                    nc.tensor.matmul(
                        psums[i],
                        statC_sb[:, kt * 64 : kt * 64 + 64],
                        hb[:, s : s + n],
                        start=False,
                        stop=False,
                    )

            for i, (s, n) in enumerate(BLKS):
                # rank-1 corrections: -zeros*rowsum and -128*sum(c')
                nc.tensor.matmul(
                    psums[i],
                    corrL_sb,
                    corrR_sb[:, s : s + n],
                    start=False,
                    stop=True,
                )
                t0 = opool.tile([B, n], dt.float32, name="t0", tag=f"t0{i}")
                t = opool.tile([B, n], dt.float32, name="t", tag=f"t{i}")
                o = opool.tile([B, n], dt.float32, name="o", tag=f"o{i}")
                # lo-group psum -> sbuf on ACT (only one psum read allowed per TT)
                nc.scalar.activation(
                    t0, psums[i][32:48, :], mybir.ActivationFunctionType.Copy
                )
                nc.vector.tensor_tensor(t, psums[i][0:16, :], t0, op.add)
                nc.vector.tensor_tensor(o, t, sc_sb[:, s : s + n], op.mult)
                nc.sync.dma_start(out[:, s : s + n], o)

    nc.finalize()
    return nc


def _get_program():
    if "nc" not in _CACHE:
        _CACHE["nc"] = _build_program()
    return _CACHE["nc"]


def _host_prep(inp, quant_weight, scales, zeros):
    """Build per-core input maps (layout/precision prep only, no dequant math)."""
    inp64 = np.asarray(inp, dtype=np.float64)
    a = inp64[:, 0::2].T.copy()  # [HALF, B] even-i activations (pair with l)
    b = inp64[:, 1::2].T.copy()  # [HALF, B] odd-i activations (pair with h)
    # q-stream pairs with a; bit-trick h-stream pairs with c' = (b-16a)/8
    cp = (b - 16.0 * a) / 8.0
    a_hi, a_lo = _split_hi_lo(a)
    c_hi, c_lo = _split_hi_lo(cp)

    statA = np.zeros((128, KT * 64), dtype=BF16)
    statC = np.zeros((128, KT * 64), dtype=BF16)
    for kt in range(KT):
        rows = slice(kt * 128, (kt + 1) * 128)
        statA[:, kt * 64 : kt * 64 + 16] = a_hi[rows]
        statA[:, kt * 64 + 32 : kt * 64 + 48] = a_lo[rows]
        statC[:, kt * 64 : kt * 64 + 16] = c_hi[rows]
        statC[:, kt * 64 + 32 : kt * 64 + 48] = c_lo[rows]

    rowsum = inp64.sum(axis=1)  # [B]
    rs_hi, rs_lo = _split_hi_lo(rowsum)
    s_c = cp.sum(axis=0)  # [B]  sum_j c'[j,b]
    sc_hi, sc_lo = _split_hi_lo(s_c)
    corrL = np.zeros((6, 64), dtype=BF16)
    corrL[0, :16] = rs_hi
    corrL[1, :16] = rs_hi
    corrL[2, :16] = rs_lo
    corrL[3, :16] = rs_lo
    corrL[4, :16] = sc_hi
    corrL[5, :16] = sc_lo

    qw = np.asarray(quant_weight)
    scales = np.asarray(scales, dtype=np.float64).reshape(-1)
    zeros = np.asarray(zeros, dtype=np.float64).reshape(-1)

    in_maps = []
    for cidx in range(NCORES):
        rows = slice(cidx * OS, (cidx + 1) * OS)
        qc = qw[rows].astype(np.uint8).T  # [HALF, OS] natural columns
        # byte-pair columns (m, 688+m) -> uint16 elements
        qc2 = np.empty((HALF, OS), dtype=np.uint8)
        qc2[:, 0::2] = qc[:, :HOS]
        qc2[:, 1::2] = qc[:, HOS:]
        qu16 = np.ascontiguousarray(qc2).view(np.uint16)  # [HALF, HOS]
        z = zeros[rows]
        z_hi, z_lo = _split_hi_lo(z)
        corrR = np.zeros((6, OS), dtype=BF16)
        corrR[0] = -z_hi
        corrR[1] = -z_lo
        corrR[2] = -z_hi
        corrR[3] = -z_lo
        corrR[4] = -128.0
        corrR[5] = -128.0
        sc_c = np.broadcast_to(scales[rows].astype(np.float32), (B, OS)).copy()
        in_maps.append(
            {
                "q": qu16,
                "statA": statA,
                "statC": statC,
                "corrL": corrL,
                "corrR": corrR,
                "sc": sc_c,
            }
        )
    return in_maps


def kernel(inp, quant_weight, scales, zeros):
    from concourse.bass_utils import run_bass_kernel_spmd

    nc = _get_program()
    in_maps = _host_prep(inp, quant_weight, scales, zeros)
    res = run_bass_kernel_spmd(nc, in_maps, core_ids=list(range(NCORES)))
    out = np.concatenate(
        [res.results[c]["out"] for c in range(NCORES)], axis=1
    )
    return np.ascontiguousarray(out.astype(np.float32))



# revision 2
# speedup vs baseline: 2.4367x; 2.4367x over previous
"""4-bit ColBlockQuantizedLinear on 8 TRN2 cores — fp8-DoubleRow scheme.

Math: out[b,o] = scales[o] * (sum_i inp[b,i]*wq[o,i] - zeros[o]*rowsum[b]),
wq packed two nibbles per byte (even i -> low nibble, odd i -> high).

Key device trick: fp8e4m3 bit patterns 0x00..0x0F encode EXACTLY n * 2^-9
(IEEE gradual underflow: denormals + first normal binade are linear in n).
So nibble extraction is just two u32 SWAR ops per chunk on DVE —
   l-plane = q & 0x0F0F0F0F,  h-plane = (q >> 4) & 0x0F0F0F0F
— whose byte outputs ARE valid fp8 encodings of nibble/512.  No casts.

Matmul: DoubleRow fp8 (2 fp8/cell, 2 elem/cycle ingest).  Moving operand =
[128, 2(plane), Ncols] nibble planes; stationary = [128, 2, 32] e4m3 hi/lo
split activations (cols 0:16 hi(b), 16:32 lo(b)); psum [32, Nblk] fp32
accumulates over 16 k-tiles exactly.  Psum is ACT/DVE-copied to SBUF and
DMA'd out as [32, 1376] fp32 per core.

Host glue (O(B*I)+O(B*O) only; all O(B*I*O) work on device): byte-layout
repack, e4m3 hi/lo split of activations, and the final combine
   out = 512*scales*(P_hi + P_lo) - scales*zeros*rowsum.

Sharding: column-parallel over out_features (1376 per core), activations
replicated; per-core [32,1376] psum images gathered and combined on host.
"""

import numpy as np
import ml_dtypes

B = 16
I = 4096
O = 11008
NCORES = 8
OS = O // NCORES          # 1376 out-features per core
HALF = I // 2             # 2048 packed byte-rows
KT = 16                   # 128-row contraction tiles
NCH = 4                   # DMA chunks
KPC = KT // NCH           # k-tiles per chunk
CHB = KPC * OS            # bytes/partition/chunk = 5504
BLKS = [(0, 512), (512, 512), (1024, 352)]

FP8 = ml_dtypes.float8_e4m3fn
BF16 = ml_dtypes.bfloat16

_CACHE = {}


def _build_program():
    import concourse.bacc as bacc
    import concourse.mybir as mybir
    import concourse.tile as tile

    dt = mybir.dt
    op = mybir.AluOpType
    nc = bacc.Bacc("TRN2", target_bir_lowering=False)

    q = nc.dram_tensor("q", [128, KT * OS], dt.uint8, kind="ExternalInput")
    stat = nc.dram_tensor("stat", [128, 2 * KT * 32], dt.uint8, kind="ExternalInput")
    out = nc.dram_tensor("out", [32, OS], dt.float32, kind="ExternalOutput")

    with tile.TileContext(nc) as tc:
        with (
            tc.tile_pool(name="consts", bufs=1) as cpool,
            tc.tile_pool(name="qp", bufs=2) as qpool,
            tc.tile_pool(name="pl", bufs=3) as plpool,
            tc.tile_pool(name="op", bufs=2) as opool,
            tc.tile_pool(name="ps", bufs=1, space="PSUM") as pspool,
        ):
            stat_sb = cpool.tile([128, 2 * KT * 32], dt.uint8, name="stat_sb")
            nc.sync.dma_start(stat_sb, stat[:, :])
            sf8 = stat_sb.bitcast(dt.float8e4)[:, :].rearrange(
                "p (s c) -> p s c", s=2
            )  # [128, 2, KT*32]

            psums = [
                pspool.tile([32, n], dt.float32, name=f"ps{i}")
                for i, (_, n) in enumerate(BLKS)
            ]

            for ch in range(NCH):
                qt = qpool.tile([128, CHB], dt.uint8, name="qt", tag="qt")
                nc.sync.dma_start(qt, q[:, ch * CHB : (ch + 1) * CHB])
                planes = plpool.tile([128, 2 * CHB], dt.uint8, name="pl", tag="pl")
                qu32 = qt.bitcast(dt.uint32)
                pu32 = planes.bitcast(dt.uint32)
                NU = CHB // 4
                nc.vector.tensor_scalar(
                    pu32[:, 0:NU], qu32[:, :], 0x0F0F0F0F, None, op.bitwise_and
                )
                nc.vector.tensor_scalar(
                    pu32[:, NU : 2 * NU], qu32[:, :], 4, 0x0F0F0F0F,
                    op.logical_shift_right, op.bitwise_and,
                )
                pf8 = planes.bitcast(dt.float8e4)[:, :].rearrange(
                    "p (s c) -> p s c", s=2
                )  # [128, 2, CHB]
                for k in range(KPC):
                    kt = ch * KPC + k
                    for i, (s, n) in enumerate(BLKS):
                        nc.tensor.matmul(
                            psums[i],
                            sf8[:, :, kt * 32 : kt * 32 + 32],
                            pf8[:, :, k * OS + s : k * OS + s + n],
                            start=(kt == 0),
                            stop=(kt == KT - 1),
                            perf_mode=mybir.MatmulPerfMode.DoubleRow,
                        )

            for i, (s, n) in enumerate(BLKS):
                ot = opool.tile([32, n], dt.float32, name=f"ot{i}", tag=f"ot{i}")
                if i == 1:
                    nc.vector.tensor_copy(ot, psums[i])
                else:
                    nc.scalar.activation(
                        ot, psums[i], mybir.ActivationFunctionType.Copy
                    )
                nc.sync.dma_start(out[:, s : s + n], ot)

    nc.finalize()
    return nc


def _get_program():
    if "nc" not in _CACHE:
        _CACHE["nc"] = _build_program()
    return _CACHE["nc"]


def _host_prep(inp, quant_weight, scales=None, zeros=None):
    """Layout/precision prep only (no dequant math)."""
    inp64 = np.asarray(inp, dtype=np.float64)
    a = inp64[:, 0::2].T  # [2048, B]  pairs low nibbles
    bo = inp64[:, 1::2].T  # [2048, B]  pairs high nibbles

    # stat[p, s, kt, 0:16] = e4m3 hi of act rows kt*128+p; [16:32] = e4m3 lo
    stat = np.zeros((128, 2, KT, 32), dtype=FP8)
    for s, arr in ((0, a), (1, bo)):
        arr_k = arr.reshape(KT, 128, B)  # [kt, p, b]
        hi = arr_k.astype(FP8)
        lo = (arr_k - hi.astype(np.float64)).astype(FP8)
        stat[:, s, :, 0:16] = hi.transpose(1, 0, 2)
        stat[:, s, :, 16:32] = lo.transpose(1, 0, 2)
    stat_u8 = np.ascontiguousarray(stat).reshape(128, 2 * KT * 32).view(np.uint8)

    qw = np.asarray(quant_weight)
    in_maps = []
    for c in range(NCORES):
        qc = qw[c * OS : (c + 1) * OS].astype(np.uint8).T  # [2048, OS] (j, o)
        q_dev = np.ascontiguousarray(
            qc.reshape(KT, 128, OS).transpose(1, 0, 2)
        ).reshape(128, KT * OS)
        in_maps.append({"q": q_dev, "stat": stat_u8})
    return in_maps


def kernel(inp, quant_weight, scales, zeros):
    from concourse.bass_utils import run_bass_kernel_spmd

    nc = _get_program()
    in_maps = _host_prep(inp, quant_weight)
    res = run_bass_kernel_spmd(nc, in_maps, core_ids=list(range(NCORES)))

    inp64 = np.asarray(inp, dtype=np.float64)
    rowsum = inp64.sum(axis=1)  # [B]
    sc = np.asarray(scales, dtype=np.float64).reshape(-1)
    zr = np.asarray(zeros, dtype=np.float64).reshape(-1)
    outs = []
    for c in range(NCORES):
        P = np.asarray(res.results[c]["out"], dtype=np.float64)  # [32, OS]
        s_c = sc[c * OS : (c + 1) * OS]
        z_c = zr[c * OS : (c + 1) * OS]
        S = (P[:B] + P[B : 2 * B]) * 512.0
        outs.append(S * s_c[None, :] - rowsum[:, None] * (s_c * z_c)[None, :])
    out = np.concatenate(outs, axis=1).astype(np.float32)
    return np.ascontiguousarray(out)
